# revision 1
# baseline (speedup 1.0000x reference)
"""v3: grouped-level tables in bf16, one gather index per partition.

Host re-layout (free): three tables so each sample needs only 3 gathered
blocks instead of 22 rows:
  Lz  [2^20, 128]  bf16 : leaf rows of W (z vectors), 256B blocks
  G1  [2^8,  1024] bf16 : levels 1..8  packed per level-8 ancestor, 2KB blocks
  G2  [2^16, 1024] bf16 : levels 9..16 packed per level-16 ancestor, 2KB blocks
  GB  [2^20, 512]  bf16 : levels 17..20 packed per level-20 node, 1KB blocks
Root level 0 = W[0] for every sample -> broadcast once on chip.

Per 128-sample tile: 3 indirect DMAs (one idx per partition), then
products + tree-add reduction on DVE, sigmoid on ACT, product over levels.
"""

import sys

for _p in ("/opt/trn_rl_repo", "/root/.axon_site/_ro/trn_rl_repo"):
    if _p not in sys.path:
        sys.path.append(_p)

import ml_dtypes
import numpy as np

import concourse.bacc as bacc
import concourse.bass as bass
import concourse.mybir as mybir
import concourse.tile as tile
from concourse.bass_utils import run_bass_kernel_spmd

N_CORES = 8
BATCH = 65536
PER_CORE = BATCH // N_CORES        # 8192
DEPTH = 20
OFFSET = (1 << DEPTH) - 1
SIZE = (1 << (DEPTH + 1)) - 1
D = 128
P = 128
TILES = PER_CORE // P              # 64
NLEV = DEPTH + 1                   # 21
NA = 8                             # levels per G1/G2 block
NB = 4                             # levels 17..20 in GB
LROWS = 1 << DEPTH                 # 2^20
AROWS = 1 << 16
G1ROWS = 1 << 8

f32 = mybir.dt.float32
bf16 = mybir.dt.bfloat16
i32 = mybir.dt.int32
bfnp = ml_dtypes.bfloat16


def prepare_tables(W: np.ndarray):
    Wb = W.astype(bfnp)
    Lz = np.ascontiguousarray(Wb[OFFSET:OFFSET + LROWS])
    # G1: row r <-> level-8 node id c8 = r + 2^8 - 1; cols [l-1] = level l
    G1 = np.empty((G1ROWS, NA * D), dtype=bfnp)
    ids = np.arange(G1ROWS, dtype=np.int64) + (G1ROWS - 1)
    for lev in range(8, 0, -1):
        G1[:, (lev - 1) * D:lev * D] = Wb[ids]
        ids = (ids - 1) >> 1
    # G2: row r <-> level-16 node id c16 = r + 2^16 - 1; cols [l-9] = level l
    G2 = np.empty((AROWS, NA * D), dtype=bfnp)
    ids = np.arange(AROWS, dtype=np.int64) + (AROWS - 1)
    for lev in range(16, 8, -1):
        G2[:, (lev - 9) * D:(lev - 8) * D] = Wb[ids]
        ids = (ids - 1) >> 1
    # GB: row r <-> level-20 node id c20 = r + 2^20 - 1; cols [l-17] = level l
    GB = np.empty((LROWS, NB * D), dtype=bfnp)
    ids = np.arange(LROWS, dtype=np.int64) + (LROWS - 1)
    for lev in range(20, 16, -1):
        GB[:, (lev - 17) * D:(lev - 16) * D] = Wb[ids]
        ids = (ids - 1) >> 1
    w0 = np.broadcast_to(Wb[0:1], (P, D)).copy()
    return Lz, G1, G2, GB, w0


def build_kernel():
    nc = bacc.Bacc("TRN2", target_bir_lowering=False, debug=False,
                   num_devices=N_CORES)

    coll = nc.dram_tensor("coll", [PER_CORE, 2], i32, kind="ExternalInput")
    Lz = nc.dram_tensor("Lz", [LROWS, D], bf16, kind="ExternalInput")
    G1 = nc.dram_tensor("G1", [G1ROWS, NA * D], bf16, kind="ExternalInput")
    G2 = nc.dram_tensor("G2", [AROWS, NA * D], bf16, kind="ExternalInput")
    GB = nc.dram_tensor("GB", [LROWS, NB * D], bf16, kind="ExternalInput")
    w0 = nc.dram_tensor("w0", [P, D], bf16, kind="ExternalInput")
    out = nc.dram_tensor("out", [PER_CORE], f32, kind="ExternalOutput")

    with tile.TileContext(nc) as tc:
        with (
            tc.tile_pool(name="const", bufs=1) as cpool,
            tc.tile_pool(name="gz", bufs=4) as zpool,
            tc.tile_pool(name="ga", bufs=4) as apool,
            tc.tile_pool(name="gb", bufs=4) as bpool,
            tc.tile_pool(name="pr", bufs=2) as ppool,
            tc.tile_pool(name="ix", bufs=4) as ipool,
        ):
            coll_sb = cpool.tile([P, TILES, 2], i32)
            nc.sync.dma_start(
                out=coll_sb[:],
                in_=coll.ap().rearrange("(p n) c -> p n c", p=P),
            )
            w0_sb = cpool.tile([P, D], bf16)
            nc.sync.dma_start(out=w0_sb[:], in_=w0.ap())

            # idx_z = col0 ; idx_1 = (b>>12) - 2^8 ; idx_a = (b>>4) - 2^16 ;
            # idx_b = b - 2^20
            idx_z = cpool.tile([P, TILES], i32)
            idx_1 = cpool.tile([P, TILES], i32)
            idx_a = cpool.tile([P, TILES], i32)
            idx_b = cpool.tile([P, TILES], i32)
            b_sb = cpool.tile([P, TILES], i32)
            nc.vector.tensor_copy(out=idx_z[:], in_=coll_sb[:, :, 0])
            nc.vector.tensor_scalar(
                out=b_sb[:], in0=coll_sb[:, :, 1],
                scalar1=OFFSET + 1, scalar2=None, op0=mybir.AluOpType.add)
            nc.vector.tensor_scalar(
                out=idx_a[:], in0=b_sb[:], scalar1=4, scalar2=None,
                op0=mybir.AluOpType.logical_shift_right)
            nc.vector.tensor_scalar(
                out=idx_a[:], in0=idx_a[:], scalar1=AROWS, scalar2=None,
                op0=mybir.AluOpType.subtract)
            nc.vector.tensor_scalar(
                out=idx_1[:], in0=b_sb[:], scalar1=12, scalar2=None,
                op0=mybir.AluOpType.logical_shift_right)
            nc.vector.tensor_scalar(
                out=idx_1[:], in0=idx_1[:], scalar1=G1ROWS, scalar2=None,
                op0=mybir.AluOpType.subtract)
            nc.vector.tensor_scalar(
                out=idx_b[:], in0=b_sb[:], scalar1=LROWS, scalar2=None,
                op0=mybir.AluOpType.subtract)

            logits = cpool.tile([P, TILES, NLEV + 3], f32)

            for n in range(TILES):
                # stage this tile's indices into dedicated offset-0 [P,1]
                # tiles -- exactly the AP shape the production scatter_add
                # gather uses (nonzero-offset idx APs misbehave on HW)
                iz = ipool.tile([P, 1], i32, tag="iz", name="iz")
                i1 = ipool.tile([P, 1], i32, tag="i1", name="i1")
                ia = ipool.tile([P, 1], i32, tag="ia", name="ia")
                ib = ipool.tile([P, 1], i32, tag="ib", name="ib")
                nc.vector.tensor_copy(out=iz[:], in_=idx_z[:, n:n + 1])
                nc.vector.tensor_copy(out=i1[:], in_=idx_1[:, n:n + 1])
                nc.vector.tensor_copy(out=ia[:], in_=idx_a[:, n:n + 1])
                nc.vector.tensor_copy(out=ib[:], in_=idx_b[:, n:n + 1])
                gz = zpool.tile([P, D], bf16, tag="gz")
                g1 = apool.tile([P, NA * D], bf16, tag="g1", name="g1")
                ga = apool.tile([P, NA * D], bf16, tag="ga")
                gb = bpool.tile([P, NB * D], bf16, tag="gb")
                nc.gpsimd.indirect_dma_start(
                    out=gz[:], out_offset=None, in_=Lz.ap(),
                    in_offset=bass.IndirectOffsetOnAxis(ap=iz[:, :1], axis=0))
                nc.gpsimd.indirect_dma_start(
                    out=g1[:], out_offset=None, in_=G1.ap(),
                    in_offset=bass.IndirectOffsetOnAxis(ap=i1[:, :1], axis=0))
                nc.gpsimd.indirect_dma_start(
                    out=ga[:], out_offset=None, in_=G2.ap(),
                    in_offset=bass.IndirectOffsetOnAxis(ap=ia[:, :1], axis=0))
                nc.gpsimd.indirect_dma_start(
                    out=gb[:], out_offset=None, in_=GB.ap(),
                    in_offset=bass.IndirectOffsetOnAxis(ap=ib[:, :1], axis=0))

                z3 = gz[:].unsqueeze(1)  # [P,1,D]
                prod = ppool.tile([P, NLEV + 3, D], bf16, tag="prod")
                # levels 1..8
                nc.vector.tensor_tensor(
                    out=prod[:, 0:NA, :],
                    in0=g1[:].rearrange("p (l d) -> p l d", d=D),
                    in1=z3.to_broadcast([P, NA, D]),
                    op=mybir.AluOpType.mult)
                # levels 9..16
                nc.vector.tensor_tensor(
                    out=prod[:, NA:2 * NA, :],
                    in0=ga[:].rearrange("p (l d) -> p l d", d=D),
                    in1=z3.to_broadcast([P, NA, D]),
                    op=mybir.AluOpType.mult)
                # levels 17..20
                nc.vector.tensor_tensor(
                    out=prod[:, 2 * NA:2 * NA + NB, :],
                    in0=gb[:].rearrange("p (l d) -> p l d", d=D),
                    in1=z3.to_broadcast([P, NB, D]),
                    op=mybir.AluOpType.mult)
                # root (level 0)
                nc.vector.tensor_tensor(
                    out=prod[:, 2 * NA + NB, :],
                    in0=gz[:], in1=w0_sb[:],
                    op=mybir.AluOpType.mult)
                # pad rows so the tree-add works on 24 rows
                nc.vector.memset(prod[:, NLEV:, :], 0.0)
                # reduce over d: 3 halvings (128->16) then tensor_reduce
                h1 = ppool.tile([P, NLEV + 3, D // 2], bf16, tag="h1")
                nc.vector.tensor_tensor(
                    out=h1[:], in0=prod[:, :, 0:D // 2],
                    in1=prod[:, :, D // 2:D], op=mybir.AluOpType.add)
                h2 = ppool.tile([P, NLEV + 3, D // 4], bf16, tag="h2")
                nc.vector.tensor_tensor(
                    out=h2[:], in0=h1[:, :, 0:D // 4],
                    in1=h1[:, :, D // 4:D // 2], op=mybir.AluOpType.add)
                h3 = ppool.tile([P, NLEV + 3, D // 8], bf16, tag="h3")
                nc.vector.tensor_tensor(
                    out=h3[:], in0=h2[:, :, 0:D // 8],
                    in1=h2[:, :, D // 8:D // 4], op=mybir.AluOpType.add)
                nc.vector.tensor_reduce(
                    out=logits[:, n, :], in_=h3[:],
                    axis=mybir.AxisListType.X, op=mybir.AluOpType.add)

            # sigmoid + product over the 21 real levels
            sig = cpool.tile([P, TILES, 32], f32)
            nc.vector.memset(sig[:], 1.0)
            nc.scalar.activation(
                out=sig[:, :, 0:NLEV],
                in_=logits[:, :, 0:NLEV],
                func=mybir.ActivationFunctionType.Sigmoid)
            cur = sig
            width = 32
            while width > 2:
                width //= 2
                nxt = cpool.tile([P, TILES, width], f32, tag=f"tree{width}",
                                 name=f"tree{width}")
                nc.vector.tensor_tensor(
                    out=nxt[:], in0=cur[:, :, 0:width],
                    in1=cur[:, :, width:2 * width], op=mybir.AluOpType.mult)
                cur = nxt
            probs = cpool.tile([P, TILES], f32)
            nc.vector.tensor_tensor(
                out=probs[:], in0=cur[:, :, 0], in1=cur[:, :, 1],
                op=mybir.AluOpType.mult)
            nc.sync.dma_start(
                out=out.ap().rearrange("(p n) -> p n", p=P),
                in_=probs[:])

    nc.compile()
    return nc


_NC_CACHE = None


def _get_nc():
    global _NC_CACHE
    if _NC_CACHE is None:
        _NC_CACHE = build_kernel()
    return _NC_CACHE


def _run(collocation: np.ndarray, W: np.ndarray, trace: bool = False,
         **spmd_kwargs):
    collocation = np.ascontiguousarray(collocation, dtype=np.int32)
    W = np.ascontiguousarray(W, dtype=np.float32)
    assert collocation.shape == (BATCH, 2)
    assert W.shape == (SIZE, D)

    # Sort samples by context vertex so each gather instruction (one
    # 128-sample tile) touches 128 consecutive sorted samples -> ascending
    # DRAM addresses in the grouped tables. Device position q = p*64+j holds
    # sorted sample j*128+p (tile j = sorted samples [j*128,(j+1)*128)).
    order = np.argsort(collocation[:, 1], kind="stable").astype(np.int64)
    coll_sorted = collocation[order]
    arr = (np.arange(TILES)[None, :] * P
           + np.arange(P)[:, None]).reshape(-1)     # q -> local sorted idx

    Lz, G1, G2, GB, w0 = prepare_tables(W)
    nc = _get_nc()
    in_maps = []
    for c in range(N_CORES):
        core_sorted = coll_sorted[c * PER_CORE:(c + 1) * PER_CORE]
        in_maps.append(
            {"coll": np.ascontiguousarray(core_sorted[arr]),
             "Lz": Lz, "G1": G1, "G2": G2, "GB": GB, "w0": w0})
    res = run_bass_kernel_spmd(
        nc, in_maps, core_ids=list(range(N_CORES)), trace=trace,
        **spmd_kwargs)
    outs = []
    for c in range(N_CORES):
        od = res.results[c]["out"].reshape(P, TILES)
        outs.append(od.T.reshape(-1))               # back to sorted order
    out_sorted = np.concatenate(outs)
    out = np.empty_like(out_sorted)
    out[order] = out_sorted
    return out, res


def kernel(collocation: np.ndarray, W: np.ndarray) -> np.ndarray:
    out, _ = _run(collocation, W, trace=False)
    return out



# revision 2
# speedup vs baseline: 1.2292x; 1.2292x over previous
"""v5: v4 + dedup of levels 1..12 via per-chunk candidate tables + PE.

Per chunk (1024 context-sorted samples) the distinct level<=12 ancestors
are few: <=16 level-8 ancestors (c8 span ~4) and <=96 level-12 ancestors
(c12 span ~64). The host ships, per chunk:
  CANDT [128d, 512]  bf16: cols 0:128   = level-l (1..8) rows of the 16
                            c8 candidates, col l*16+k, TRANSPOSED (d on
                            partitions); cols 128:512 = levels 9..12 of
                            the 96 c12 candidates, col 128 + l*96 + k.
  OH8   [P, CT, 16]  f32 one-hot of (c8 - c8_min) per sample
  OH12  [P, CT, 96]  f32 one-hot of (c12 - c12_min) per sample

On device, per tile: zT = PE-transpose(gz); B = zT.T @ CANDT (PSUM
[s, 512] = all candidate logits); DVE one-hot-select + reduce gives
logits for levels 1..12. Levels 13..16 (L1316 1KB rows), 17..20 (GB) and
z stay as per-sample multi-offset indirect gathers; root on-chip.

DMA/core: z 2MB + L1316 8MB + GB 8MB + cands ~1MB (vs 43MB in v4).
"""

import sys

for _p in ("/opt/trn_rl_repo", "/root/.axon_site/_ro/trn_rl_repo"):
    if _p not in sys.path:
        sys.path.append(_p)

import ml_dtypes
import numpy as np

import concourse.bacc as bacc
import concourse.bass as bass
import concourse.mybir as mybir
import concourse.tile as tile
from concourse.bass_utils import run_bass_kernel_spmd
from concourse.masks import make_identity

N_CORES = 8
BATCH = 65536
PER_CORE = BATCH // N_CORES        # 8192
DEPTH = 20
OFFSET = (1 << DEPTH) - 1
SIZE = (1 << (DEPTH + 1)) - 1
D = 128
P = 128
TILES = PER_CORE // P              # 64
CT = 8                             # tiles per chunk
NCH = TILES // CT                  # 8 chunks
LROWS = 1 << DEPTH
C16ROWS = 1 << 16
K8 = 16                            # c8 candidates per chunk
K12 = 96                           # c12 candidates per chunk
NCOLS = 8 * K8 + 4 * K12           # 512

f32 = mybir.dt.float32
bf16 = mybir.dt.bfloat16
i32 = mybir.dt.int32
bfnp = ml_dtypes.bfloat16


def prepare_tables(W: np.ndarray):
    Wb = W.astype(bfnp)
    Lz = np.ascontiguousarray(Wb[OFFSET:OFFSET + LROWS])
    # L1316: row r <-> level-16 node id r + 2^16 - 1; cols [l-13] = level l
    L1316 = np.empty((C16ROWS, 4 * D), dtype=bfnp)
    ids = np.arange(C16ROWS, dtype=np.int64) + (C16ROWS - 1)
    for lev in range(16, 12, -1):
        L1316[:, (lev - 13) * D:(lev - 12) * D] = Wb[ids]
        ids = (ids - 1) >> 1
    GB = np.empty((LROWS, 4 * D), dtype=bfnp)
    ids = np.arange(LROWS, dtype=np.int64) + (LROWS - 1)
    for lev in range(20, 16, -1):
        GB[:, (lev - 17) * D:(lev - 16) * D] = Wb[ids]
        ids = (ids - 1) >> 1
    w0 = np.broadcast_to(Wb[0:1], (P, D)).copy()
    return Lz, L1316, GB, w0


def build_kernel():
    nc = bacc.Bacc("TRN2", target_bir_lowering=False, debug=False,
                   num_devices=N_CORES)

    Lz = nc.dram_tensor("Lz", [LROWS, D], bf16, kind="ExternalInput")
    L1316 = nc.dram_tensor("L1316", [C16ROWS, 4 * D], bf16,
                           kind="ExternalInput")
    GB = nc.dram_tensor("GB", [LROWS, 4 * D], bf16, kind="ExternalInput")
    w0 = nc.dram_tensor("w0", [P, D], bf16, kind="ExternalInput")
    CANDT = nc.dram_tensor("CANDT", [NCH, P, NCOLS], bf16,
                           kind="ExternalInput")
    OH8 = nc.dram_tensor("OH8", [NCH, P, CT * K8], f32,
                         kind="ExternalInput")
    OH12 = nc.dram_tensor("OH12", [NCH, P, CT * K12], f32,
                          kind="ExternalInput")
    IZ = nc.dram_tensor("IZ", [NCH, P, CT], i32, kind="ExternalInput")
    I16 = nc.dram_tensor("I16", [NCH, P, CT], i32, kind="ExternalInput")
    IB = nc.dram_tensor("IB", [NCH, P, CT], i32, kind="ExternalInput")
    out = nc.dram_tensor("out", [PER_CORE], f32, kind="ExternalOutput")

    with tile.TileContext(nc) as tc:
        with (
            tc.tile_pool(name="const", bufs=1) as cpool,
            tc.tile_pool(name="ix", bufs=4) as ipool,
            tc.tile_pool(name="gz", bufs=2) as zpool,
            tc.tile_pool(name="gl", bufs=2) as lpool,
            tc.tile_pool(name="gb", bufs=2) as bpool,
            tc.tile_pool(name="cand", bufs=2) as candpool,
            tc.tile_pool(name="pr", bufs=1) as ppool,
            tc.tile_pool(name="zt", bufs=3) as ztpool,
            tc.tile_pool(name="pst", bufs=2,
                         space=bass.MemorySpace.PSUM) as pstpool,
            tc.tile_pool(name="psb", bufs=2,
                         space=bass.MemorySpace.PSUM) as psbpool,
        ):
            w0_sb = cpool.tile([P, D], bf16)
            nc.sync.dma_start(out=w0_sb[:], in_=w0.ap())
            ident = cpool.tile([P, P], bf16)
            make_identity(nc, ident[:])
            logits = cpool.tile([P, TILES, 24], f32)

            for c in range(NCH):
                iz = ipool.tile([P, CT], i32, tag="iz")
                il = ipool.tile([P, CT], i32, tag="il")
                ib = ipool.tile([P, CT], i32, tag="ib")
                nc.sync.dma_start(out=iz[:], in_=IZ.ap()[c])
                nc.sync.dma_start(out=il[:], in_=I16.ap()[c])
                nc.sync.dma_start(out=ib[:], in_=IB.ap()[c])
                candt = candpool.tile([P, NCOLS], bf16, tag="candt")
                oh8 = candpool.tile([P, CT, K8], f32, tag="oh8")
                oh12 = candpool.tile([P, CT, K12], f32, tag="oh12")
                nc.sync.dma_start(out=candt[:], in_=CANDT.ap()[c])
                nc.sync.dma_start(
                    out=oh8[:],
                    in_=OH8.ap()[c].rearrange("p (t k) -> p t k", k=K8))
                nc.sync.dma_start(
                    out=oh12[:],
                    in_=OH12.ap()[c].rearrange("p (t k) -> p t k", k=K12))

                gz = zpool.tile([P, CT, D], bf16, tag="gz")
                gl = lpool.tile([P, CT, 4 * D], bf16, tag="gl")
                gb = bpool.tile([P, CT, 4 * D], bf16, tag="gb")
                # per-tile [P,1] single-offset gathers (multi-offset idx APs
                # misbehave on HW); idx staged into offset-0 [P,1] tiles
                for t in range(CT):
                    izt = ipool.tile([P, 1], i32, tag="izt", name="izt")
                    ilt = ipool.tile([P, 1], i32, tag="ilt", name="ilt")
                    ibt = ipool.tile([P, 1], i32, tag="ibt", name="ibt")
                    nc.vector.tensor_copy(out=izt[:], in_=iz[:, t:t + 1])
                    nc.vector.tensor_copy(out=ilt[:], in_=il[:, t:t + 1])
                    nc.vector.tensor_copy(out=ibt[:], in_=ib[:, t:t + 1])
                    nc.gpsimd.indirect_dma_start(
                        out=gz[:, t, :], out_offset=None, in_=Lz.ap(),
                        in_offset=bass.IndirectOffsetOnAxis(
                            ap=izt[:, :1], axis=0))
                    nc.gpsimd.indirect_dma_start(
                        out=gl[:, t, :], out_offset=None, in_=L1316.ap(),
                        in_offset=bass.IndirectOffsetOnAxis(
                            ap=ilt[:, :1], axis=0))
                    nc.gpsimd.indirect_dma_start(
                        out=gb[:, t, :], out_offset=None, in_=GB.ap(),
                        in_offset=bass.IndirectOffsetOnAxis(
                            ap=ibt[:, :1], axis=0))

                # per-tile: zT, all-candidate logits, one-hot select
                for t in range(CT):
                    n = c * CT + t
                    psT = pstpool.tile([P, P], bf16, tag="psT")
                    nc.tensor.transpose(
                        out=psT[:], in_=gz[:, t, :], identity=ident[:])
                    zT = ztpool.tile([P, P], bf16, tag="zT")
                    nc.scalar.copy(out=zT[:], in_=psT[:])
                    psB = psbpool.tile([P, NCOLS], f32, tag="psB")
                    nc.tensor.matmul(
                        out=psB[:], lhsT=zT[:], rhs=candt[:],
                        start=True, stop=True)
                    sel8 = ztpool.tile([P, 8, K8], f32, tag="sel8")
                    nc.vector.tensor_tensor(
                        out=sel8[:],
                        in0=psB[:, 0:8 * K8].rearrange(
                            "p (l k) -> p l k", k=K8),
                        in1=oh8[:, t].unsqueeze(1).to_broadcast([P, 8, K8]),
                        op=mybir.AluOpType.mult)
                    nc.vector.tensor_reduce(
                        out=logits[:, n, 1:9], in_=sel8[:],
                        axis=mybir.AxisListType.X, op=mybir.AluOpType.add)
                    sel12 = ztpool.tile([P, 4, K12], f32, tag="sel12")
                    nc.vector.tensor_tensor(
                        out=sel12[:],
                        in0=psB[:, 8 * K8:NCOLS].rearrange(
                            "p (l k) -> p l k", k=K12),
                        in1=oh12[:, t].unsqueeze(1).to_broadcast(
                            [P, 4, K12]),
                        op=mybir.AluOpType.mult)
                    nc.vector.tensor_reduce(
                        out=logits[:, n, 9:13], in_=sel12[:],
                        axis=mybir.AxisListType.X, op=mybir.AluOpType.add)

                z4 = gz[:].unsqueeze(2)              # [P, CT, 1, D]
                # levels 13..16
                pl = ppool.tile([P, CT, 4, D], bf16, tag="pl")
                nc.vector.tensor_tensor(
                    out=pl[:],
                    in0=gl[:].rearrange("p a (l d) -> p a l d", d=D),
                    in1=z4.to_broadcast([P, CT, 4, D]),
                    op=mybir.AluOpType.mult)
                hl1 = ppool.tile([P, CT, 4, D // 2], bf16, tag="hl1")
                nc.vector.tensor_tensor(
                    out=hl1[:], in0=pl[:, :, :, 0:D // 2],
                    in1=pl[:, :, :, D // 2:D], op=mybir.AluOpType.add)
                hl2 = ppool.tile([P, CT, 4, D // 4], bf16, tag="hl2")
                nc.vector.tensor_tensor(
                    out=hl2[:], in0=hl1[:, :, :, 0:D // 4],
                    in1=hl1[:, :, :, D // 4:D // 2], op=mybir.AluOpType.add)
                hl3 = ppool.tile([P, CT, 4, D // 8], bf16, tag="hl3")
                nc.vector.tensor_tensor(
                    out=hl3[:], in0=hl2[:, :, :, 0:D // 8],
                    in1=hl2[:, :, :, D // 8:D // 4], op=mybir.AluOpType.add)
                nc.vector.tensor_reduce(
                    out=logits[:, c * CT:(c + 1) * CT, 13:17], in_=hl3[:],
                    axis=mybir.AxisListType.X, op=mybir.AluOpType.add)

                # levels 17..20
                pb = ppool.tile([P, CT, 4, D], bf16, tag="pb")
                nc.vector.tensor_tensor(
                    out=pb[:],
                    in0=gb[:].rearrange("p a (l d) -> p a l d", d=D),
                    in1=z4.to_broadcast([P, CT, 4, D]),
                    op=mybir.AluOpType.mult)
                hb1 = ppool.tile([P, CT, 4, D // 2], bf16, tag="hb1")
                nc.vector.tensor_tensor(
                    out=hb1[:], in0=pb[:, :, :, 0:D // 2],
                    in1=pb[:, :, :, D // 2:D], op=mybir.AluOpType.add)
                hb2 = ppool.tile([P, CT, 4, D // 4], bf16, tag="hb2")
                nc.vector.tensor_tensor(
                    out=hb2[:], in0=hb1[:, :, :, 0:D // 4],
                    in1=hb1[:, :, :, D // 4:D // 2], op=mybir.AluOpType.add)
                hb3 = ppool.tile([P, CT, 4, D // 8], bf16, tag="hb3")
                nc.vector.tensor_tensor(
                    out=hb3[:], in0=hb2[:, :, :, 0:D // 8],
                    in1=hb2[:, :, :, D // 8:D // 4], op=mybir.AluOpType.add)
                nc.vector.tensor_reduce(
                    out=logits[:, c * CT:(c + 1) * CT, 17:21], in_=hb3[:],
                    axis=mybir.AxisListType.X, op=mybir.AluOpType.add)

                # root (level 0)
                pr = ppool.tile([P, CT, D], bf16, tag="pr")
                nc.vector.tensor_tensor(
                    out=pr[:], in0=gz[:],
                    in1=w0_sb[:].unsqueeze(1).to_broadcast([P, CT, D]),
                    op=mybir.AluOpType.mult)
                hr1 = ppool.tile([P, CT, D // 2], bf16, tag="hr1")
                nc.vector.tensor_tensor(
                    out=hr1[:], in0=pr[:, :, 0:D // 2],
                    in1=pr[:, :, D // 2:D], op=mybir.AluOpType.add)
                hr2 = ppool.tile([P, CT, D // 4], bf16, tag="hr2")
                nc.vector.tensor_tensor(
                    out=hr2[:], in0=hr1[:, :, 0:D // 4],
                    in1=hr1[:, :, D // 4:D // 2], op=mybir.AluOpType.add)
                hr3 = ppool.tile([P, CT, D // 8], bf16, tag="hr3")
                nc.vector.tensor_tensor(
                    out=hr3[:], in0=hr2[:, :, 0:D // 8],
                    in1=hr2[:, :, D // 8:D // 4], op=mybir.AluOpType.add)
                nc.vector.tensor_reduce(
                    out=logits[:, c * CT:(c + 1) * CT, 0:1].rearrange(
                        "p a x -> p (a x)"),
                    in_=hr3[:], axis=mybir.AxisListType.X,
                    op=mybir.AluOpType.add)

            sig = cpool.tile([P, TILES, 32], f32)
            nc.vector.memset(sig[:], 1.0)
            nc.scalar.activation(
                out=sig[:, :, 0:21], in_=logits[:, :, 0:21],
                func=mybir.ActivationFunctionType.Sigmoid)
            cur = sig
            width = 32
            while width > 2:
                width //= 2
                nxt = cpool.tile([P, TILES, width], f32, tag=f"tree{width}",
                                 name=f"tree{width}")
                nc.vector.tensor_tensor(
                    out=nxt[:], in0=cur[:, :, 0:width],
                    in1=cur[:, :, width:2 * width], op=mybir.AluOpType.mult)
                cur = nxt
            probs = cpool.tile([P, TILES], f32)
            nc.vector.tensor_tensor(
                out=probs[:], in0=cur[:, :, 0], in1=cur[:, :, 1],
                op=mybir.AluOpType.mult)
            nc.sync.dma_start(
                out=out.ap().rearrange("(p n) -> p n", p=P),
                in_=probs[:])

    nc.compile()
    return nc


_NC_CACHE = None


def _get_nc():
    global _NC_CACHE
    if _NC_CACHE is None:
        _NC_CACHE = build_kernel()
    return _NC_CACHE


def make_core_inputs(collocation: np.ndarray, Wb: np.ndarray):
    """Sort by context; per-core idx arrays + candidate tables."""
    order = np.argsort(collocation[:, 1], kind="stable").astype(np.int64)
    coll_sorted = collocation[order]
    core_inputs = []
    for c in range(N_CORES):
        cs = coll_sorted[c * PER_CORE:(c + 1) * PER_CORE]
        leaf = cs[:, 0].astype(np.int64)
        ctx = cs[:, 1].astype(np.int64)
        b = ctx + (1 << DEPTH)
        i16v = (b >> 4) - C16ROWS

        def lay(v):
            return np.ascontiguousarray(
                v.reshape(TILES, P).T.reshape(P, NCH, CT)
                .transpose(1, 0, 2)).astype(np.int32)

        # candidates per chunk
        c8 = (b >> 12) - 256          # [8192] in [0, 256)
        c12 = (b >> 8) - 4096         # [8192] in [0, 4096)
        candT = np.empty((NCH, P, NCOLS), dtype=bfnp)
        oh8 = np.zeros((NCH, P, CT, K8), dtype=np.float32)
        oh12 = np.zeros((NCH, P, CT, K12), dtype=np.float32)
        for ch in range(NCH):
            sl = slice(ch * CT * P, (ch + 1) * CT * P)
            c8c, c12c = c8[sl], c12[sl]
            b8, b12 = int(c8c.min()), int(c12c.min())
            assert int(c8c.max()) - b8 < K8, "c8 span exceeds K8"
            assert int(c12c.max()) - b12 < K12, "c12 span exceeds K12"
            # cand rows, levels 1..8 for c8 cands / 9..12 for c12 cands
            ids8 = np.minimum(b8 + np.arange(K8), 255) + 255
            block8 = np.empty((K8, 8, D), dtype=bfnp)
            for lev in range(8, 0, -1):
                block8[:, lev - 1] = Wb[ids8]
                ids8 = (ids8 - 1) >> 1
            ids12 = np.minimum(b12 + np.arange(K12), 4095) + 4095
            block12 = np.empty((K12, 4, D), dtype=bfnp)
            for lev in range(12, 8, -1):
                block12[:, lev - 9] = Wb[ids12]
                ids12 = (ids12 - 1) >> 1
            # candT[d, l*K + k]
            candT[ch, :, 0:8 * K8] = block8.transpose(2, 1, 0).reshape(
                D, 8 * K8)
            candT[ch, :, 8 * K8:] = block12.transpose(2, 1, 0).reshape(
                D, 4 * K12)
            # one-hots, slot (p, t) = chunk sample t*128 + p
            k8v = (c8c - b8).reshape(CT, P)     # [t, p]
            k12v = (c12c - b12).reshape(CT, P)
            tt, pp = np.meshgrid(np.arange(CT), np.arange(P),
                                 indexing="ij")
            oh8[ch, pp.ravel(), tt.ravel(), k8v.ravel()] = 1.0
            oh12[ch, pp.ravel(), tt.ravel(), k12v.ravel()] = 1.0
        core_inputs.append({
            "IZ": lay(leaf), "I16": lay(i16v), "IB": lay(ctx),
            "CANDT": candT,
            "OH8": np.ascontiguousarray(
                oh8.transpose(0, 1, 2, 3).reshape(NCH, P, CT * K8)),
            "OH12": np.ascontiguousarray(
                oh12.reshape(NCH, P, CT * K12))})
    return order, core_inputs


def _run(collocation: np.ndarray, W: np.ndarray, trace: bool = False,
         **spmd_kwargs):
    collocation = np.ascontiguousarray(collocation, dtype=np.int32)
    W = np.ascontiguousarray(W, dtype=np.float32)
    assert collocation.shape == (BATCH, 2)
    assert W.shape == (SIZE, D)

    Lz, L1316, GB, w0 = prepare_tables(W)
    Wb = W.astype(bfnp)
    order, core_inputs = make_core_inputs(collocation, Wb)
    nc = _get_nc()
    in_maps = []
    for c in range(N_CORES):
        m = {"Lz": Lz, "L1316": L1316, "GB": GB, "w0": w0}
        m.update(core_inputs[c])
        in_maps.append(m)
    res = run_bass_kernel_spmd(
        nc, in_maps, core_ids=list(range(N_CORES)), trace=trace,
        **spmd_kwargs)
    outs = []
    for c in range(N_CORES):
        od = res.results[c]["out"].reshape(P, TILES)
        outs.append(od.T.reshape(-1))
    out_sorted = np.concatenate(outs)
    out = np.empty_like(out_sorted)
    out[order] = out_sorted
    return out, res


def kernel(collocation: np.ndarray, W: np.ndarray) -> np.ndarray:
    out, _ = _run(collocation, W, trace=False)
    return out
